# revision 1
# baseline (speedup 1.0000x reference)
"""CRF-as-RNN mean-field kernel for Trainium2 (Bass/Tile), 8-core SPMD.

Strategy:
- Shard 2 images x 4 row-strips across 8 cores. Each core gets 84 rows
  (64 owned + halo); 5 mean-field iterations shrink the valid region by
  2 rows/iter, so no inter-core communication is needed at all.
- On-chip layout: partitions = 6 row-groups x 21 channels = 126; free dim
  = 14 rows x 256 cols (+2-row/-col halos for in-tile shifted reads:
  18 row-slots x 260 col-slots). Image-boundary zero padding is realized
  by statically-zero halo slots; intra-core group halos are refreshed
  once per iteration with two SBUF->SBUF DMAs.
- The 5x5 spatial gaussian (sigma=0.1) is a numerical delta in f32, so
  sp == q; it is folded into the center-tap constant.
- Bilateral 24-tap MAC runs on DVE in fp16 (2x mode where aligned),
  using 12 unique weight maps (opposite taps share maps by symmetry).
- Softmax runs chunked through PSUM: z = logits - compat-transform via
  PE matmuls; exp/ln on ACT; normalization via the exp(z - lnD) trick
  (lnD broadcast back into PSUM by a mask matmul) - no division needed.
- Bilateral color weights are precomputed once: diff/square on DVE/ACT,
  3-channel reduction + 21-channel broadcast via PE mask matmuls,
  exp(-50*d^2 + ln(spatial)) on ACT.
"""

import math
import sys
from contextlib import ExitStack

import numpy as np

sys.path.insert(0, "/opt/trn_rl_repo")

# ---------------- problem constants (hardcoded per contract) ----------------
B, C, H, W = 2, 21, 256, 256
G, RG = 6, 14                  # row groups per strip, rows per group
P = G * C                      # 126 partitions
F = RG * W                     # 3584 free elems (real pixels per partition)
NT, NV = 18, 260               # q/w tile row slots (-2..15), col slots (-2..257)
IU, IV = 22, 264               # img tile row slots (-4..17), col slots (-4..259)
STARTS = [0, 54, 118, 172]     # strip start rows
OWN = [(0, 64), (10, 74), (10, 74), (20, 84)]  # owned local-row range per strip
NUM_ITERS = 5
NCH, CH = 7, 512               # softmax chunks (512 px = 2 rows)
NPC, CP = 10, 468              # w-precompute chunks over NT*NV=4680

# spatial gaussian (5x5, sigma=5), normalized
_ax = np.arange(5, dtype=np.float64) - 2
_xx, _yy = np.meshgrid(_ax, _ax, indexing="ij")
_g = np.exp(-(_xx**2 + _yy**2) / (2 * 5.0**2))
SW = (_g / _g.sum()).astype(np.float64)
WC = float(SW[2, 2])           # center weight (spatial only; color=1 at center)
# 12 unique taps (positive half-window); opposite taps share weight maps
TAPS = [(0, 1), (0, 2), (1, -2), (1, -1), (1, 0), (1, 1), (1, 2),
        (2, -2), (2, -1), (2, 0), (2, 1), (2, 2)]

_BASS_CACHE = {}


def _build_bass():
    import concourse.bass as bass
    import concourse.mybir as mybir
    from concourse import tile

    f32 = mybir.dt.float32
    f16 = mybir.dt.float16
    AF = mybir.ActivationFunctionType
    OP = mybir.AluOpType

    nc = bass.Bass("TRN2", target_bir_lowering=False, debug=False,
                   enable_asserts=False)

    lg_d = nc.dram_tensor("lg", [P, F], f32, kind="ExternalInput")
    img_d = nc.dram_tensor("img", [18, IU * IV], f32, kind="ExternalInput")
    mneg_d = nc.dram_tensor("mneg", [P, P], f16, kind="ExternalInput")
    iden_d = nc.dram_tensor("iden", [P, P], f32, kind="ExternalInput")
    onesd_d = nc.dram_tensor("onesd", [P, G], f16, kind="ExternalInput")
    chmask_d = nc.dram_tensor("chmask", [18, G], f16, kind="ExternalInput")
    bneg_d = nc.dram_tensor("bneg", [G, P], f32, kind="ExternalInput")
    bpos_d = nc.dram_tensor("bpos", [G, P * 12], f32, kind="ExternalInput")
    qout_d = nc.dram_tensor("qout", [P, F], f32, kind="ExternalOutput")

    with tile.TileContext(nc) as tc, ExitStack() as ctx:
        const_pool = ctx.enter_context(tc.tile_pool(name="const", bufs=1))
        main_pool = ctx.enter_context(tc.tile_pool(name="main", bufs=1))
        w_pool = ctx.enter_context(tc.tile_pool(name="wmaps", bufs=1))

        mneg_t = const_pool.tile([P, P], f16, tag="mneg")
        nc.sync.dma_start(mneg_t[:], mneg_d.ap())
        iden_t = const_pool.tile([P, P], f32, tag="iden")
        nc.sync.dma_start(iden_t[:], iden_d.ap())
        onesd_t = const_pool.tile([P, G], f16, tag="onesd")
        nc.sync.dma_start(onesd_t[:], onesd_d.ap())
        chmask_t = const_pool.tile([18, G], f16, tag="chmask")
        nc.sync.dma_start(chmask_t[:], chmask_d.ap())
        bneg_t = const_pool.tile([G, P], f32, tag="bneg")
        nc.sync.dma_start(bneg_t[:], bneg_d.ap())
        bpos_t = const_pool.tile([G, P * 12], f32, tag="bpos")
        nc.sync.dma_start(bpos_t[:], bpos_d.ap())

        # Absorber matmuls: each PE matmul can carry only ~1 sync wait
        # beyond its own-engine wait, so pre-observe every stationary's DMA
        # queue with a 2-column dummy matmul (self-referential rhs => the
        # dummy itself waits on exactly one DMA sem).
        with tc.tile_pool(name="scrp", bufs=1, space="PSUM") as scrp:
            scr = scrp.tile([G, 2], f32, tag="scr")
            nc.tensor.matmul(scr[:1, :], mneg_t[:, 0:1], mneg_t[:, 0:2],
                             start=True, stop=True)
            nc.tensor.matmul(scr[:1, :], iden_t[:, 0:1], iden_t[:, 0:2],
                             start=True, stop=True)
            nc.tensor.matmul(scr[:, :], onesd_t[:], onesd_t[:, 0:2],
                             start=True, stop=True)
            nc.tensor.matmul(scr[:, :], chmask_t[:], chmask_t[:, 0:2],
                             start=True, stop=True)
            nc.tensor.matmul(scr[:1, :], bneg_t[:, 0:1], bneg_t[:, 0:2],
                             start=True, stop=True)
            nc.tensor.matmul(scr[:1, :], bpos_t[:, 0:1], bpos_t[:, 0:2],
                             start=True, stop=True)

        q_t = main_pool.tile([P, NT * NV], f16, tag="q")
        nc.vector.memset(q_t[:], 0.0)
        q3 = q_t[:].rearrange("p (t v) -> p t v", v=NV)


        w_tiles = [w_pool.tile([P, NT * NV], f16, tag=f"w{i}", name=f"w{i}")
                   for i in range(len(TAPS))]

        zps_pool = ctx.enter_context(tc.tile_pool(name="zps", bufs=3,
                                                  space="PSUM"))
        dps_pool = ctx.enter_context(tc.tile_pool(name="dps", bufs=1,
                                                  space="PSUM"))

        # ---------------- w-map precompute ----------------
        with tc.tile_pool(name="pre", bufs=1) as prep, \
             tc.tile_pool(name="pre2", bufs=3) as prep2, \
             tc.tile_pool(name="psp", bufs=2, space="PSUM") as psp, \
             tc.tile_pool(name="psw", bufs=2, space="PSUM") as psw:
            img_t = prep.tile([18, IU * IV], f32, tag="img")
            nc.sync.dma_start(img_t[:], img_d.ap())
            img3 = img_t[:].rearrange("p (u v) -> p u v", v=IV)
            diff_t = prep.tile([18, NT * NV], f16, tag="diff")
            diff3 = diff_t[:].rearrange("p (t v) -> p t v", v=NV)
            sq_t = prep.tile([18, NT * NV], f16, tag="sq")

            for ki, (dy, dx) in enumerate(TAPS):
                nc.vector.tensor_sub(
                    diff3[:, 0:NT, 0:NV],
                    img3[:, 2 + dy:2 + dy + NT, 2 + dx:2 + dx + NV],
                    img3[:, 2:2 + NT, 2:2 + NV],
                )
                nc.scalar.square(sq_t[:], diff_t[:])
                for cc in range(NPC):
                    sl = slice(cc * CP, (cc + 1) * CP)
                    d2_ps = psp.tile([G, CP], f32, tag="d2")
                    nc.tensor.matmul(d2_ps[:], chmask_t[:],
                                     sq_t[:, sl], start=True, stop=True)
                    e6 = prep2.tile([G, CP], f32, tag="e6")
                    nc.scalar.activation(e6[:], d2_ps[:], AF.Exp,
                                         scale=-50.0)
                    w_ps = psw.tile([P, CP], f32, tag="wps")
                    nc.tensor.matmul(w_ps[:],
                                     bpos_t[:, ki * P:(ki + 1) * P], e6[:],
                                     start=True, stop=True)
                    nc.scalar.copy(w_tiles[ki][:, sl], w_ps[:])

        # ---------------- iteration tiles ----------------
        post_pool = ctx.enter_context(tc.tile_pool(name="post", bufs=1))
        lg_t = post_pool.tile([P, F], f32, tag="lg")
        nc.sync.dma_start(lg_t[:], lg_d.ap())
        lg2_t = post_pool.tile([P, F], f32, tag="lg2")
        nc.scalar.copy(lg2_t[:], lg_t[:])  # ACT-owned copy for PE reads
        acc_t = post_pool.tile([P, F], f16, tag="acc")
        acc3 = acc_t[:].rearrange("p (r x) -> p r x", x=W)
        tmp_pool = ctx.enter_context(tc.tile_pool(name="tmp", bufs=2))
        e_pool = ctx.enter_context(tc.tile_pool(name="E", bufs=2))
        ln_pool = ctx.enter_context(tc.tile_pool(name="ln", bufs=2))

        def softmax_pass(with_s: bool, last: bool):
            for c in range(NCH):
                sl = slice(c * CH, (c + 1) * CH)
                z_ps = zps_pool.tile([P, CH], f32, tag="z")
                if with_s:
                    nc.tensor.matmul(z_ps[:], mneg_t[:], acc_t[:, sl],
                                     start=True, stop=False)
                    nc.tensor.matmul(z_ps[:], iden_t[:], lg2_t[:, sl],
                                     start=False, stop=False,
                                     skip_group_check=True)
                else:
                    nc.tensor.matmul(z_ps[:], iden_t[:], lg2_t[:, sl],
                                     start=True, stop=False,
                                     skip_group_check=True)
                e_t = e_pool.tile([P, CH], f16, tag="E")
                nc.scalar.activation(e_t[:], z_ps[:], AF.Exp)
                d_ps = dps_pool.tile([G, CH], f32, tag="D")
                nc.tensor.matmul(d_ps[:], onesd_t[:], e_t[:],
                                 start=True, stop=True)
                ln_t = ln_pool.tile([G, CH], f32, tag="ln")
                nc.scalar.activation(ln_t[:], d_ps[:], AF.Ln)
                nc.tensor.matmul(z_ps[:], bneg_t[:], ln_t[:],
                                 start=False, stop=True,
                                 skip_group_check=True)
                z3 = z_ps[:].rearrange("p (r x) -> p r x", x=W)
                if last:
                    lg3 = lg_t[:].rearrange("p (r x) -> p r x", x=W)
                    nc.scalar.activation(lg3[:, 2 * c:2 * c + 2, 0:W],
                                         z3, AF.Exp)
                else:
                    nc.scalar.activation(
                        q3[:, 2 + 2 * c:4 + 2 * c, 2:2 + W], z3, AF.Exp)

        softmax_pass(with_s=False, last=False)   # q0 = softmax(logits)

        for it in range(NUM_ITERS):
            last = it == NUM_ITERS - 1
            # refresh intra-core group halos (2 SBUF->SBUF DMAs)
            nc.sync.dma_start(q3[21:126, 0:2, 0:NV], q3[0:105, 14:16, 0:NV])
            nc.sync.dma_start(q3[0:105, 16:18, 0:NV], q3[21:126, 2:4, 0:NV])

            # bilateral: 24 taps = 12 unique maps x {gather, scatter-sym}
            first = True
            for ki, (dy, dx) in enumerate(TAPS):
                w3 = w_tiles[ki][:].rearrange("p (t v) -> p t v", v=NV)
                for (qdy, qdx, wdy, wdx) in ((dy, dx, 0, 0),
                                             (-dy, -dx, -dy, -dx)):
                    q_ap = q3[:, 2 + qdy:2 + qdy + RG, 2 + qdx:2 + qdx + W]
                    w_ap = w3[:, 2 + wdy:2 + wdy + RG, 2 + wdx:2 + wdx + W]
                    if first:
                        nc.vector.tensor_mul(acc3[:, 0:RG, 0:W], q_ap, w_ap)
                        first = False
                    else:
                        t = tmp_pool.tile([P, F], f16, tag="tmp")
                        t3 = t[:].rearrange("p (r x) -> p r x", x=W)
                        nc.vector.tensor_mul(t3[:, 0:RG, 0:W], q_ap, w_ap)
                        nc.vector.tensor_add(acc_t[:], acc_t[:], t[:])

            # acc = q*(1+wc) + acc   (spatial delta-conv fold, in place)
            nc.vector.scalar_tensor_tensor(
                acc3[:, 0:RG, 0:W], q3[:, 2:2 + RG, 2:2 + W],
                float(1.0 + WC), acc3[:, 0:RG, 0:W],
                OP.mult, OP.add)

            softmax_pass(with_s=True, last=last)

        nc.sync.dma_start(qout_d.ap(), lg_t[:])

    _legalize_matmul_waits(nc, mybir)
    return nc


def _legalize_matmul_waits(nc, mybir, max_waits=2):
    """TRN2 ISA sync-wait structs hold few waits per instruction (2 for PE
    matmult/NoOp, 1 for DVE TensorTensor, ...); codegen aborts on more.
    Move excess waits onto InstNoOps (1 wait each) inserted right before
    on the same engine (adjacent => identical blocking semantics)."""
    cap = {}
    for f in nc.m.functions:
        for blk in f.blocks:
            insts = blk.instructions
            out = []
            changed = False
            for i in insts:
                si = getattr(i, "sync_info", None)
                eng = getattr(i, "engine", None)
                max_waits = cap.get(type(i).__name__, 1)
                if (si is not None and eng is not None
                        and len(si.on_wait) > max_waits):
                    waits = list(si.on_wait)
                    keep, move = [], []
                    for w in waits:
                        if "PE" in w.ant_name and len(keep) < max_waits:
                            keep.append(w)
                        else:
                            move.append(w)
                    while len(keep) < max_waits and move:
                        keep.append(move.pop())
                    nop_cap = cap.get("InstNoOp", 1)
                    while move:
                        grp, move = move[:nop_cap], move[nop_cap:]
                        nop = mybir.InstNoOp(
                            name=nc.get_next_instruction_name(),
                            engine=eng, ins=[], outs=[])
                        nop.sync_info = mybir.SyncInfo(on_wait=grp,
                                                       on_update=[])
                        out.append(nop)
                    i.sync_info = mybir.SyncInfo(
                        on_wait=keep, on_update=list(si.on_update))
                    changed = True
                out.append(i)
            if changed:
                blk.instructions = out


def _prep_shards(logits, img, compat):
    """Host-side shard prep -> list of 8 in_maps."""
    mneg = np.kron(np.eye(G), -compat.T.astype(np.float64)).astype(np.float16)
    iden = np.eye(P, dtype=np.float32)
    onesd = np.kron(np.eye(G), np.ones((C, 1))).astype(np.float16)
    chmask = np.kron(np.eye(G), np.ones((3, 1))).astype(np.float16)
    bneg = np.kron(np.eye(G), -np.ones((1, C))).astype(np.float32)
    bpos = np.concatenate(
        [np.kron(np.eye(G), float(SW[2 + dy, 2 + dx]) * np.ones((1, C)))
         for (dy, dx) in TAPS], axis=1).astype(np.float32)

    in_maps = []
    for core in range(8):
        b, j = divmod(core, 4)
        s = STARTS[j]
        lg = logits[b, :, s:s + 84, :].reshape(C, G, RG, W)
        lg = np.ascontiguousarray(
            lg.transpose(1, 0, 2, 3).reshape(P, F)).astype(np.float32)
        im = np.zeros((G, 3, IU, IV), np.float32)
        for g in range(G):
            base = s + g * RG - 4
            u0, u1 = max(0, -base), min(IU, H - base)
            im[g, :, u0:u1, 4:4 + W] = img[b, :, base + u0:base + u1, :]
        im = im.reshape(18, IU * IV)
        in_maps.append({
            "lg": lg, "img": np.ascontiguousarray(im),
            "mneg": mneg, "iden": iden, "onesd": onesd,
            "chmask": chmask, "bneg": bneg, "bpos": bpos,
        })
    return in_maps


def kernel(**inputs):
    logits = np.asarray(inputs["logits"], dtype=np.float32)
    img = np.asarray(inputs["img"], dtype=np.float32)
    compat = np.asarray(inputs["compat_mat"], dtype=np.float32)

    from concourse.bass_utils import run_bass_kernel_spmd

    if "nc" not in _BASS_CACHE:
        _BASS_CACHE["nc"] = _build_bass()
    nc = _BASS_CACHE["nc"]

    in_maps = _prep_shards(logits, img, compat)
    res = run_bass_kernel_spmd(nc, in_maps, core_ids=list(range(8)))
    _BASS_CACHE["last_result"] = res

    out = np.zeros((B, C, H, W), np.float32)
    for core in range(8):
        b, j = divmod(core, 4)
        s = STARTS[j]
        lo, hi = OWN[j]
        qc = res.results[core]["qout"].reshape(G, C, RG, W)
        qc = qc.transpose(1, 0, 2, 3).reshape(C, 84, W)
        out[b, :, s + lo:s + hi, :] = qc[:, lo:hi, :]
    return out



# revision 5
# speedup vs baseline: 1.4149x; 1.4149x over previous
"""CRF-as-RNN mean-field kernel for Trainium2 (Bass/Tile), 8-core SPMD.

Strategy:
- Shard 2 images x 4 row-strips across 8 cores. Each core gets 84 rows
  (64 owned + halo); 5 mean-field iterations shrink the valid region by
  2 rows/iter, so no inter-core communication is needed at all.
- On-chip layout: partitions = 6 row-groups x 21 channels = 126; free dim
  = 14 rows x 256 cols (+2-row/-col halos for in-tile shifted reads:
  18 row-slots x 260 col-slots). Image-boundary zero padding is realized
  by statically-zero halo slots; intra-core group halos are refreshed
  once per iteration with two SBUF->SBUF DMAs.
- The 5x5 spatial gaussian (sigma=0.1) is a numerical delta in f32, so
  sp == q; it is folded into the compat matmul: z += (1+wc)*mneg @ q.
- Bilateral 24-tap MAC runs on DVE in fp16 (2x mode), using 12 unique
  weight maps (opposite taps share maps by symmetry).
- w-map precompute: img arrives f16; diff/square on DVE at 2x; a single
  [18->126] f16 mask matmul kron(I6, ones(3,21)) does the 3-channel
  reduction AND the 21-channel broadcast in one op; ACT exp applies
  scale=-50 and folds the spatial weight via bias=ln(s_k).
- Softmax chunked through PSUM, all-f16 matmuls: z = mneg@acc +
  (1+wc)*mneg@q + I@lg; exp on ACT; D via ones-mask matmul; lnD
  recentered by ln(21) (scale=1/21) so f16 holds it accurately; the
  -lnD broadcast back into PSUM by a mask matmul; final exp carries
  bias=-ln(21). Output written f16, host upcasts.
"""

import math
import sys
from contextlib import ExitStack

import numpy as np

sys.path.insert(0, "/opt/trn_rl_repo")

# ---------------- problem constants (hardcoded per contract) ----------------
B, C, H, W = 2, 21, 256, 256
G, RG = 6, 14                  # row groups per strip, rows per group
P = G * C                      # 126 partitions
F = RG * W                     # 3584 free elems (real pixels per partition)
NT, NV = 18, 260               # q/w tile row slots (-2..15), col slots (-2..257)
IU, IV = 22, 264               # img tile row slots (-4..17), col slots (-4..259)
STARTS = [0, 54, 118, 172]     # strip start rows
OWN = [(0, 64), (10, 74), (10, 74), (20, 84)]  # owned local-row range per strip
NUM_ITERS = 5
NCH, CH = 7, 512               # softmax chunks (512 px = 2 rows)
NPC, CP = 10, 468              # w-precompute chunks over NT*NV=4680
LN21 = math.log(21.0)

# spatial gaussian (5x5, sigma=5), normalized
_ax = np.arange(5, dtype=np.float64) - 2
_xx, _yy = np.meshgrid(_ax, _ax, indexing="ij")
_g = np.exp(-(_xx**2 + _yy**2) / (2 * 5.0**2))
SW = (_g / _g.sum()).astype(np.float64)
WC = float(SW[2, 2])           # center weight (spatial only; color=1 at center)
# 12 unique taps (positive half-window); opposite taps share weight maps
TAPS = [(0, 1), (0, 2), (1, -2), (1, -1), (1, 0), (1, 1), (1, 2),
        (2, -2), (2, -1), (2, 0), (2, 1), (2, 2)]

_BASS_CACHE = {}


def _build_bass():
    import concourse.bass as bass
    import concourse.mybir as mybir
    from concourse import tile

    f32 = mybir.dt.float32
    f16 = mybir.dt.float16
    AF = mybir.ActivationFunctionType
    OP = mybir.AluOpType

    nc = bass.Bass("TRN2", target_bir_lowering=False, debug=False,
                   enable_asserts=False)

    lg_d = nc.dram_tensor("lg", [P, F], f16, kind="ExternalInput")
    img_d = nc.dram_tensor("img", [18, IU * IV], f16, kind="ExternalInput")
    mneg_d = nc.dram_tensor("mneg", [P, P], f16, kind="ExternalInput")
    mneg2_d = nc.dram_tensor("mneg2", [P, P], f16, kind="ExternalInput")
    iden_d = nc.dram_tensor("iden", [P, P], f16, kind="ExternalInput")
    onesd_d = nc.dram_tensor("onesd", [P, G], f16, kind="ExternalInput")
    bneg_d = nc.dram_tensor("bneg", [G, P], f16, kind="ExternalInput")
    m18_d = nc.dram_tensor("m18", [18, P], f16, kind="ExternalInput")
    lnsw_d = nc.dram_tensor("lnsw", [P, 13], f32, kind="ExternalInput")
    qout_d = nc.dram_tensor("qout", [P, F], f16, kind="ExternalOutput")

    with tile.TileContext(nc) as tc, ExitStack() as ctx:
        const_pool = ctx.enter_context(tc.tile_pool(name="const", bufs=1))
        main_pool = ctx.enter_context(tc.tile_pool(name="main", bufs=1))
        w_pool = ctx.enter_context(tc.tile_pool(name="wmaps", bufs=1))

        mneg_t = const_pool.tile([P, P], f16, tag="mneg")
        nc.sync.dma_start(mneg_t[:], mneg_d.ap())
        mneg2_t = const_pool.tile([P, P], f16, tag="mneg2")
        nc.sync.dma_start(mneg2_t[:], mneg2_d.ap())
        iden_t = const_pool.tile([P, P], f16, tag="iden")
        nc.sync.dma_start(iden_t[:], iden_d.ap())
        onesd_t = const_pool.tile([P, G], f16, tag="onesd")
        nc.sync.dma_start(onesd_t[:], onesd_d.ap())
        bneg_t = const_pool.tile([G, P], f16, tag="bneg")
        nc.sync.dma_start(bneg_t[:], bneg_d.ap())
        m18_t = const_pool.tile([18, P], f16, tag="m18")
        nc.sync.dma_start(m18_t[:], m18_d.ap())
        lnsw_t = const_pool.tile([P, 13], f32, tag="lnsw")
        nc.sync.dma_start(lnsw_t[:], lnsw_d.ap())

        # Absorber matmuls: each PE matmul can carry only ~1 sync wait
        # beyond its own-engine wait, so pre-observe every stationary's DMA
        # queue with a 2-column dummy matmul (self-referential rhs => the
        # dummy itself waits on exactly one DMA sem).
        with tc.tile_pool(name="scrp", bufs=1, space="PSUM") as scrp:
            scr = scrp.tile([G, 2], f32, tag="scr")
            nc.tensor.matmul(scr[:1, :], mneg_t[:, 0:1], mneg_t[:, 0:2],
                             start=True, stop=True)
            nc.tensor.matmul(scr[:1, :], mneg2_t[:, 0:1], mneg2_t[:, 0:2],
                             start=True, stop=True)
            nc.tensor.matmul(scr[:1, :], iden_t[:, 0:1], iden_t[:, 0:2],
                             start=True, stop=True)
            nc.tensor.matmul(scr[:, :], onesd_t[:], onesd_t[:, 0:2],
                             start=True, stop=True)
            nc.tensor.matmul(scr[:1, :], bneg_t[:, 0:1], bneg_t[:, 0:2],
                             start=True, stop=True)
            nc.tensor.matmul(scr[:1, :], m18_t[:, 0:1], m18_t[:, 0:2],
                             start=True, stop=True)

        q_t = main_pool.tile([P, NT * NV], f16, tag="q")
        nc.vector.memset(q_t[:], 0.0)
        q3 = q_t[:].rearrange("p (t v) -> p t v", v=NV)

        w_tiles = [w_pool.tile([P, NT * NV], f16, tag=f"w{i}", name=f"w{i}")
                   for i in range(len(TAPS))]

        zps_pool = ctx.enter_context(tc.tile_pool(name="zps", bufs=3,
                                                  space="PSUM"))
        dps_pool = ctx.enter_context(tc.tile_pool(name="dps", bufs=2,
                                                  space="PSUM"))

        # ---------------- w-map precompute ----------------
        with tc.tile_pool(name="pre", bufs=1) as prep, \
             tc.tile_pool(name="psp", bufs=3, space="PSUM") as psp:
            img_t = prep.tile([18, IU * IV], f16, tag="img")
            nc.sync.dma_start(img_t[:], img_d.ap())
            img3 = img_t[:].rearrange("p (u v) -> p u v", v=IV)
            diff_t = prep.tile([18, NT * NV], f16, tag="diff")
            diff3 = diff_t[:].rearrange("p (t v) -> p t v", v=NV)
            sq_t = prep.tile([18, NT * NV], f16, tag="sq")

            for ki, (dy, dx) in enumerate(TAPS):
                nc.vector.tensor_sub(
                    diff3[:, 0:NT, 0:NV],
                    img3[:, 2 + dy:2 + dy + NT, 2 + dx:2 + dx + NV],
                    img3[:, 2:2 + NT, 2:2 + NV],
                )
                nc.vector.tensor_mul(sq_t[:], diff_t[:], diff_t[:])
                for cc in range(NPC):
                    sl = slice(cc * CP, (cc + 1) * CP)
                    d2_ps = psp.tile([P, CP], f32, tag="d2")
                    nc.tensor.matmul(d2_ps[:], m18_t[:], sq_t[:, sl],
                                     start=True, stop=True)
                    nc.scalar.activation(w_tiles[ki][:, sl], d2_ps[:],
                                         AF.Exp, scale=-50.0,
                                         bias=lnsw_t[:, ki:ki + 1])

        # ---------------- iteration tiles ----------------
        post_pool = ctx.enter_context(tc.tile_pool(name="post", bufs=1))
        lg_t = post_pool.tile([P, F], f16, tag="lg")
        nc.sync.dma_start(lg_t[:], lg_d.ap())
        qo_t = post_pool.tile([P, F], f16, tag="qo")
        qo3 = qo_t[:].rearrange("p (r x) -> p r x", x=W)
        acc_t = post_pool.tile([P, F], f16, tag="acc")
        acc3 = acc_t[:].rearrange("p (r x) -> p r x", x=W)
        tmp_pool = ctx.enter_context(tc.tile_pool(name="tmp", bufs=2))
        e_pool = ctx.enter_context(tc.tile_pool(name="E", bufs=2))
        ln_pool = ctx.enter_context(tc.tile_pool(name="ln", bufs=2))

        def softmax_pass(with_s: bool, last: bool):
            for c in range(NCH):
                sl = slice(c * CH, (c + 1) * CH)
                z_ps = zps_pool.tile([P, CH], f32, tag="z")
                if with_s:
                    nc.tensor.matmul(z_ps[:], mneg_t[:], acc_t[:, sl],
                                     start=True, stop=False)
                    nc.tensor.matmul(z_ps[:], mneg2_t[:],
                                     q3[:, 2 + 2 * c:4 + 2 * c, 2:2 + W],
                                     start=False, stop=False,
                                     skip_group_check=True)
                    nc.tensor.matmul(z_ps[:], iden_t[:], lg_t[:, sl],
                                     start=False, stop=False,
                                     skip_group_check=True)
                else:
                    nc.tensor.matmul(z_ps[:], iden_t[:], lg_t[:, sl],
                                     start=True, stop=False,
                                     skip_group_check=True)
                e_t = e_pool.tile([P, CH], f16, tag="E")
                nc.scalar.activation(e_t[:], z_ps[:], AF.Exp)
                d_ps = dps_pool.tile([G, CH], f32, tag="D")
                nc.tensor.matmul(d_ps[:], onesd_t[:], e_t[:],
                                 start=True, stop=True)
                ln_t = ln_pool.tile([G, CH], f16, tag="ln")
                # lnD - ln21 stays ~O(1) => accurate in f16
                nc.scalar.activation(ln_t[:], d_ps[:], AF.Ln,
                                     scale=float(1.0 / 21.0))
                nc.tensor.matmul(z_ps[:], bneg_t[:], ln_t[:],
                                 start=False, stop=True,
                                 skip_group_check=True)
                z3 = z_ps[:].rearrange("p (r x) -> p r x", x=W)
                if last:
                    nc.scalar.activation(qo3[:, 2 * c:2 * c + 2, 0:W],
                                         z3, AF.Exp,
                                         bias=lnsw_t[:, 12:13])
                else:
                    nc.scalar.activation(
                        q3[:, 2 + 2 * c:4 + 2 * c, 2:2 + W], z3, AF.Exp,
                        bias=lnsw_t[:, 12:13])

        softmax_pass(with_s=False, last=False)   # q0 = softmax(logits)

        for it in range(NUM_ITERS):
            last = it == NUM_ITERS - 1
            # refresh intra-core group halos (2 SBUF->SBUF DMAs)
            nc.sync.dma_start(q3[21:126, 0:2, 0:NV], q3[0:105, 14:16, 0:NV])
            nc.sync.dma_start(q3[0:105, 16:18, 0:NV], q3[21:126, 2:4, 0:NV])

            # bilateral: 24 taps = 12 unique maps x {gather, scatter-sym}
            first = True
            for ki, (dy, dx) in enumerate(TAPS):
                w3 = w_tiles[ki][:].rearrange("p (t v) -> p t v", v=NV)
                for (qdy, qdx, wdy, wdx) in ((dy, dx, 0, 0),
                                             (-dy, -dx, -dy, -dx)):
                    q_ap = q3[:, 2 + qdy:2 + qdy + RG, 2 + qdx:2 + qdx + W]
                    w_ap = w3[:, 2 + wdy:2 + wdy + RG, 2 + wdx:2 + wdx + W]
                    if first:
                        nc.vector.tensor_mul(acc3[:, 0:RG, 0:W], q_ap, w_ap)
                        first = False
                    else:
                        t = tmp_pool.tile([P, F], f16, tag="tmp")
                        t3 = t[:].rearrange("p (r x) -> p r x", x=W)
                        nc.vector.tensor_mul(t3[:, 0:RG, 0:W], q_ap, w_ap)
                        nc.vector.tensor_add(acc_t[:], acc_t[:], t[:])

            softmax_pass(with_s=True, last=last)

        nc.sync.dma_start(qout_d.ap(), qo_t[:])

    _legalize_matmul_waits(nc, mybir)
    return nc


def _legalize_matmul_waits(nc, mybir, max_waits=2):
    """TRN2 ISA sync-wait structs hold few waits per instruction (2 for PE
    matmult/NoOp, 1 for DVE TensorTensor, ...); codegen aborts on more.
    Move excess waits onto InstNoOps (1 wait each) inserted right before
    on the same engine (adjacent => identical blocking semantics)."""
    cap = {}
    for f in nc.m.functions:
        for blk in f.blocks:
            insts = blk.instructions
            out = []
            changed = False
            for i in insts:
                si = getattr(i, "sync_info", None)
                eng = getattr(i, "engine", None)
                max_waits = cap.get(type(i).__name__, 1)
                if (si is not None and eng is not None
                        and len(si.on_wait) > max_waits):
                    waits = list(si.on_wait)
                    keep, move = [], []
                    for w in waits:
                        if "PE" in w.ant_name and len(keep) < max_waits:
                            keep.append(w)
                        else:
                            move.append(w)
                    while len(keep) < max_waits and move:
                        keep.append(move.pop())
                    nop_cap = cap.get("InstNoOp", 1)
                    while move:
                        grp, move = move[:nop_cap], move[nop_cap:]
                        nop = mybir.InstNoOp(
                            name=nc.get_next_instruction_name(),
                            engine=eng, ins=[], outs=[])
                        nop.sync_info = mybir.SyncInfo(on_wait=grp,
                                                       on_update=[])
                        out.append(nop)
                    i.sync_info = mybir.SyncInfo(
                        on_wait=keep, on_update=list(si.on_update))
                    changed = True
                out.append(i)
            if changed:
                blk.instructions = out


def _prep_shards(logits, img, compat):
    """Host-side shard prep -> list of 8 in_maps."""
    mneg = np.kron(np.eye(G), -compat.T.astype(np.float64)).astype(np.float16)
    mneg2 = ((1.0 + WC) *
             np.kron(np.eye(G), -compat.T.astype(np.float64))
             ).astype(np.float16)
    iden = np.eye(P, dtype=np.float16)
    onesd = np.kron(np.eye(G), np.ones((C, 1))).astype(np.float16)
    bneg = np.kron(np.eye(G), -np.ones((1, C))).astype(np.float16)
    m18 = np.kron(np.eye(G), np.ones((3, C))).astype(np.float16)
    lnsw = np.zeros((P, 13), np.float32)
    for ki, (dy, dx) in enumerate(TAPS):
        lnsw[:, ki] = math.log(SW[2 + dy, 2 + dx])
    lnsw[:, 12] = -LN21

    in_maps = []
    for core in range(8):
        b, j = divmod(core, 4)
        s = STARTS[j]
        lg = logits[b, :, s:s + 84, :].reshape(C, G, RG, W)
        lg = np.ascontiguousarray(
            lg.transpose(1, 0, 2, 3).reshape(P, F)).astype(np.float16)
        im = np.zeros((G, 3, IU, IV), np.float16)
        for g in range(G):
            base = s + g * RG - 4
            u0, u1 = max(0, -base), min(IU, H - base)
            im[g, :, u0:u1, 4:4 + W] = img[b, :, base + u0:base + u1, :]
        im = im.reshape(18, IU * IV)
        in_maps.append({
            "lg": lg, "img": np.ascontiguousarray(im),
            "mneg": mneg, "mneg2": mneg2, "iden": iden, "onesd": onesd,
            "bneg": bneg, "m18": m18, "lnsw": lnsw,
        })
    return in_maps


def kernel(**inputs):
    logits = np.asarray(inputs["logits"], dtype=np.float32)
    img = np.asarray(inputs["img"], dtype=np.float32)
    compat = np.asarray(inputs["compat_mat"], dtype=np.float32)

    from concourse.bass_utils import run_bass_kernel_spmd

    if "nc" not in _BASS_CACHE:
        _BASS_CACHE["nc"] = _build_bass()
    nc = _BASS_CACHE["nc"]

    in_maps = _prep_shards(logits, img, compat)
    res = run_bass_kernel_spmd(nc, in_maps, core_ids=list(range(8)))
    _BASS_CACHE["last_result"] = res

    out = np.zeros((B, C, H, W), np.float32)
    for core in range(8):
        b, j = divmod(core, 4)
        s = STARTS[j]
        lo, hi = OWN[j]
        qc = res.results[core]["qout"].astype(np.float32).reshape(G, C, RG, W)
        qc = qc.transpose(1, 0, 2, 3).reshape(C, 84, W)
        out[b, :, s + lo:s + hi, :] = qc[:, lo:hi, :]
    return out
